# revision 11
# baseline (speedup 1.0000x reference)
"""Trainium2 Bass kernel for nn_ModelNew_3556232921872 (dense_cnn).

Pipeline per sample:
  x_conv = conv3x3(x, W) + b
  acc    = depthwise3x3(x_conv, diag(W)) + b
  group stats over channels per pixel -> norm = (acc - mean_c) * rsqrt(var+eps)
  norm = norm * gamma + beta
  fused = tanh(norm) * clip(norm/6 + 0.5, 0, 1)
  out   = logsumexp(x_conv + fused, channels)          # [1, H, W]

Sharding: data-parallel over batch, B=16 -> 2 samples per NeuronCore x 8.

Implementation notes:
 - conv as 6 matmul passes: 3 passes K=128 (tap pairs (dy,0)+(dy,1) via a
   column-shifted second SBUF copy of the input) + 3 passes K=64 (taps (dy,2)),
   accumulated in PSUM. Depthwise conv: 9 passes K=64 with diagonal lhsT; the
   per-pixel channel-group mean is folded into the center tap
   (diag(wd) - G/8), so the DW psum directly yields t1 = acc - mean_c + bias.
 - Single ACT table set (natural_log_exp_and_others): rsqrt(v)=exp(-0.5 ln v),
   tanh(x)=1-2/(1+exp(2x)) with reciprocal_approx_fast on DVE, final ln on ACT.
"""
import numpy as np

import concourse.bass as bass
import concourse.bacc as bacc
import concourse.mybir as mybir
from concourse.tile import TileContext
from concourse.bass_utils import run_bass_kernel_spmd
from concourse.mybir import AluOpType, ActivationFunctionType

F32 = mybir.dt.float32
AF = ActivationFunctionType

B, C, H, W = 16, 64, 256, 256
K = 3
G = 8
GS = C // G
EPS = 1e-05
NCORES = 8
BPC = B // NCORES          # samples per core

R = 4                      # output rows per block
WP = W + 4                 # padded input row width (2 left, 2 right)
WO = W + 2                 # conv output row width (x in [-1, W+1))
NBLK = H // R

MAX_N_F32 = 512            # fp32 moving-operand limit per matmul


def _build_nc():
    nc = bacc.Bacc("TRN2", target_bir_lowering=False)
    x = nc.dram_tensor("x", [BPC, 128, H + 4, WP], F32, kind="ExternalInput")
    wp = nc.dram_tensor("wp", [128, 3, C], F32, kind="ExternalInput")
    ws = nc.dram_tensor("ws", [64, 3, C], F32, kind="ExternalInput")
    wd = nc.dram_tensor("wd", [64, 9, C], F32, kind="ExternalInput")
    wstat = nc.dram_tensor("wstat", [128, 128], F32, kind="ExternalInput")
    ones = nc.dram_tensor("ones", [64, 1], F32, kind="ExternalInput")
    cb = nc.dram_tensor("cb", [64, 1], F32, kind="ExternalInput")
    gam = nc.dram_tensor("gam", [64, 1], F32, kind="ExternalInput")
    bet = nc.dram_tensor("bet", [64, 1], F32, kind="ExternalInput")
    out = nc.dram_tensor("out", [BPC, H * W], F32, kind="ExternalOutput")

    NRI = R + 4            # input rows per block (conv computes R+2 out rows)
    NRO = R + 2            # conv out rows [-1, R+1)
    NPIX = R * W

    with TileContext(nc) as tc:
        with tc.tile_pool(name="consts", bufs=1) as consts, \
             tc.tile_pool(name="xin_p", bufs=2) as xin_p, \
             tc.tile_pool(name="xcv_p", bufs=2) as xcv_p, \
             tc.tile_pool(name="work", bufs=2) as work, \
             tc.tile_pool(name="outp", bufs=2) as outp, \
             tc.tile_pool(name="psbig", bufs=1, space="PSUM") as psbig, \
             tc.tile_pool(name="psa", bufs=2, space="PSUM") as psa:

            wpt = consts.tile([128, 3, C], F32)
            wst = consts.tile([64, 3, C], F32)
            wdt = consts.tile([64, 9, C], F32)
            wstt = consts.tile([128, 128], F32)
            onest = consts.tile([64, 1], F32)
            cbt = consts.tile([64, 1], F32)
            gamt = consts.tile([64, 1], F32)
            bett = consts.tile([64, 1], F32)
            nc.sync.dma_start(out=wpt, in_=wp[:, :, :])
            nc.sync.dma_start(out=wst, in_=ws[:, :, :])
            nc.sync.dma_start(out=wdt, in_=wd[:, :, :])
            nc.sync.dma_start(out=wstt, in_=wstat[:, :])
            nc.sync.dma_start(out=onest, in_=ones[:, :])
            nc.sync.dma_start(out=cbt, in_=cb[:, :])
            nc.sync.dma_start(out=gamt, in_=gam[:, :])
            nc.sync.dma_start(out=bett, in_=bet[:, :])

            for b in range(BPC):
                for blk in range(NBLK):
                    y0 = blk * R
                    # ---- input tile [128, NRI, WP]; bottom half col-shifted +1
                    xin = xin_p.tile([128, NRI, WP], F32, tag="xin")
                    nc.sync.dma_start(out=xin, in_=x[b, :, y0:y0 + NRI, :])

                    # ---- conv psum [64, NRO, W]: rows y in [y0-1, y0+R+1),
                    # x in [0, W) only (column halos of xcv are SAME-pad zeros)
                    pc = psbig.tile([64, NRO, W], F32, tag="psbig")
                    for dy in range(3):
                        for j in range(0, NRO, 2):
                            rhs = bass.AP(
                                tensor=xin.tensor,
                                offset=xin.offset + (dy + j) * WP + 2,
                                ap=[[xin.ap[0][0], 128], [WP, 2], [1, W]])
                            nc.tensor.matmul(pc[:, j:j + 2, :], wpt[:, dy, :],
                                             rhs, start=(dy == 0), stop=False)
                    for dy in range(3):
                        for j in range(0, NRO, 2):
                            rhs = bass.AP(
                                tensor=xin.tensor,
                                offset=xin.offset + (dy + j) * WP + 3,
                                ap=[[xin.ap[0][0], 64], [WP, 2], [1, W]])
                            nc.tensor.matmul(pc[:, j:j + 2, :], wst[:, dy, :],
                                             rhs, start=False,
                                             stop=(dy == 2 and j == NRO - 2))

                    # ---- xcv [128, NRO, WO]: top = x_conv (zero halo), bottom = x_conv^2
                    xcv = xcv_p.tile([128, NRO, WO], F32, tag="xcv")
                    nc.vector.memset(xcv[0:64, :, 0:1], 0.0)
                    nc.vector.memset(xcv[0:64, :, WO - 1:WO], 0.0)
                    j_lo = 1 if blk == 0 else 0
                    j_hi = R + 1 if blk == NBLK - 1 else R + 2
                    if j_lo > 0:
                        nc.vector.memset(xcv[0:64, 0:j_lo, 1:WO - 1], 0.0)
                    if j_hi < NRO:
                        nc.vector.memset(xcv[0:64, j_hi:NRO, 1:WO - 1], 0.0)
                    nc.scalar.activation(xcv[0:64, j_lo:j_hi, 1:WO - 1],
                                         pc[:, j_lo:j_hi, :], AF.Identity,
                                         bias=cbt, scale=1.0)
                    nc.scalar.activation(xcv[64:128], xcv[0:64], AF.Square)

                    # ---- depthwise psum -> t1 = acc - mean_c (center folded)
                    pd = psa.tile([64, R, W], F32, tag="psa")
                    t = 0
                    for dy in range(3):
                        for dx in range(3):
                            for j0r in range(0, R, 2):
                                rhs = bass.AP(
                                    tensor=xcv.tensor,
                                    offset=xcv.offset + (dy + j0r) * WO + dx,
                                    ap=[[xcv.ap[0][0], 64], [WO, 2], [1, W]])
                                nc.tensor.matmul(
                                    pd[:, j0r:j0r + 2, :], wdt[:, t, :], rhs,
                                    start=(t == 0),
                                    stop=(t == 8 and j0r == R - 2))
                            t += 1
                    t1 = work.tile([64, R, W], F32, tag="t1")
                    nc.scalar.activation(t1, pd, AF.Identity, bias=cbt, scale=1.0)

                    # ---- stats psum [128, R, W]: mean_c / meansq_c
                    pstat = psbig.tile([128, R, W], F32, tag="psbig")
                    for j0r in range(0, R, 2):
                        rhs = bass.AP(
                            tensor=xcv.tensor,
                            offset=xcv.offset + (1 + j0r) * WO + 1,
                            ap=[[xcv.ap[0][0], 128], [WO, 2], [1, W]])
                        nc.tensor.matmul(pstat[:, j0r:j0r + 2, :], wstt, rhs,
                                         start=True, stop=(j0r == R - 2))
                                        # inv_std = exp(-0.5 * ln(meansq - mean^2 + eps))
                    m2 = work.tile([64, R, W], F32, tag="m2")
                    nc.scalar.activation(m2, pstat[0:64], AF.Square)
                    veps = work.tile([64, R, W], F32, tag="veps")
                    nc.vector.scalar_tensor_tensor(
                        out=veps, in0=pstat[64:128], scalar=EPS, in1=m2,
                        op0=AluOpType.add, op1=AluOpType.subtract)
                    lnv = work.tile([64, R, W], F32, tag="lnv")
                    nc.scalar.activation(lnv, veps, AF.Ln)
                    isd = work.tile([64, R, W], F32, tag="isd")
                    nc.scalar.activation(isd, lnv, AF.Exp, scale=-0.5)

                    # ---- norm
                    nrm = work.tile([64, R, W], F32, tag="nrm")
                    nc.vector.tensor_tensor(nrm, t1, isd, op=AluOpType.mult)
                    nc.vector.tensor_scalar(nrm, nrm, gamt, bett,
                                            AluOpType.mult, AluOpType.add)
                    nc.vector.tensor_scalar(nrm, nrm, -30.0, 30.0,
                                            AluOpType.max, AluOpType.min)

                    # gate = clip(nrm/6 + 0.5, 0, 1)
                    gate = work.tile([64, R, W], F32, tag="gate")
                    nc.vector.tensor_scalar(gate, nrm, 1.0 / 6.0, 0.5,
                                            AluOpType.mult, AluOpType.add)
                    nc.vector.tensor_scalar(gate, gate, 0.0, 1.0,
                                            AluOpType.max, AluOpType.min)

                    # tanh(nrm) = 1 - 2/(1 + exp(2*nrm))
                    ee = work.tile([64, R, W], F32, tag="ee")
                    nc.scalar.activation(ee, nrm, AF.Exp, scale=2.0)
                    nc.vector.tensor_scalar(ee, ee, 1.0, None, AluOpType.add)
                    rr = work.tile([64, R, W], F32, tag="rr")
                    nc.vector.reciprocal_approx_fast(rr, ee)
                    # fused = (1 - 2 r) * gate = -2*(r*gate) + gate
                    nc.vector.tensor_tensor(rr, rr, gate, op=AluOpType.mult)
                    zz = work.tile([64, R, W], F32, tag="zz")
                    nc.vector.scalar_tensor_tensor(
                        out=zz, in0=rr, scalar=-2.0, in1=gate,
                        op0=AluOpType.mult, op1=AluOpType.add)
                    # z = x_conv + fused
                    xcv_int = bass.AP(
                        tensor=xcv.tensor, offset=xcv.offset + WO + 1,
                        ap=[[xcv.ap[0][0], 64], [WO, R], [1, W]])
                    nc.vector.tensor_tensor(zz, zz, xcv_int,
                                            op=AluOpType.add)
                    nc.scalar.activation(zz, zz, AF.Exp)

                    # ---- logsumexp: PE channel sum then ln
                    pl = psa.tile([1, R, W], F32, tag="psa")
                    for j0r in range(0, R, 2):
                        nc.tensor.matmul(
                            pl[:, j0r:j0r + 2, :], onest,
                            zz[:, j0r:j0r + 2, :],
                            start=True, stop=(j0r == R - 2))
                    lse = outp.tile([1, R, W], F32, tag="lse")
                    nc.scalar.activation(lse, pl, AF.Ln)
                    nc.sync.dma_start(
                        out=out[b, y0 * W:(y0 + R) * W].rearrange("(o a c) -> o a c", o=1, c=W),
                        in_=lse)
    nc.compile()
    return nc


def _host_weights(conv_w, conv_b, gn_scale, gn_bias):
    w = np.asarray(conv_w, np.float32)
    wp = np.stack([np.concatenate([w[:, :, dy, 1].T, w[:, :, dy, 0].T], axis=0)
                   for dy in range(3)], axis=1).astype(np.float32)
    ws = np.stack([w[:, :, dy, 2].T for dy in range(3)], axis=1).astype(np.float32)

    wdiag = np.einsum('cckl->ckl', w)                       # [C, 3, 3]
    gsel = np.zeros((C, C), np.float32)
    for g in range(G):
        gsel[g * GS:(g + 1) * GS, g * GS:(g + 1) * GS] = 1.0 / GS
    dmats = []
    for dy in range(3):
        for dx in range(3):
            m = np.diag(wdiag[:, dy, dx]).astype(np.float32)
            if dy == 1 and dx == 1:
                m = m - gsel            # fold -mean_c (lhsT[ci,co]: G sym)
            dmats.append(m)
    wd = np.stack(dmats, axis=1).astype(np.float32)          # [64, 9, 64]

    wstat = np.zeros((128, 128), np.float32)
    wstat[0:64, 0:64] = gsel
    wstat[64:128, 64:128] = gsel

    ones = np.ones((64, 1), np.float32)
    cb = np.asarray(conv_b, np.float32).reshape(64, 1)
    gam = np.asarray(gn_scale, np.float32).reshape(64, 1)
    bet = np.asarray(gn_bias, np.float32).reshape(64, 1)
    return dict(wp=wp, ws=ws, wd=wd, wstat=wstat, ones=ones,
                cb=cb, gam=gam, bet=bet)


_NC_CACHE = None


def kernel(x, conv_w, conv_b, gn_scale, gn_bias):
    global _NC_CACHE
    x = np.asarray(x, np.float32)
    wts = _host_weights(conv_w, conv_b, gn_scale, gn_bias)
    if _NC_CACHE is None:
        _NC_CACHE = _build_nc()
    nc = _NC_CACHE
    xpad = np.zeros((B, 128, H + 4, WP), np.float32)
    xpad[:, 0:64, 2:2 + H, 2:2 + W] = x
    xpad[:, 64:128, 2:2 + H, 3:3 + W] = x
    in_maps = []
    for c in range(NCORES):
        m = {"x": np.ascontiguousarray(xpad[c * BPC:(c + 1) * BPC])}
        m.update(wts)
        in_maps.append(m)
    import os as _os
    trace = bool(int(_os.environ.get("KTRACE", "0")))
    res = run_bass_kernel_spmd(nc, in_maps, core_ids=list(range(NCORES)),
                               trace=trace)
    kernel.exec_time_ns = res.exec_time_ns
    kernel.results_obj = res
    outs = [res.results[c]["out"].reshape(BPC, 1, H, W) for c in range(NCORES)]
    return np.concatenate(outs, axis=0)


if __name__ == "__main__":
    rng = np.random.default_rng(0)
    xs = rng.standard_normal((B, C, H, W), dtype=np.float32)
    wv = (rng.standard_normal((C, C, K, K), dtype=np.float32)
          / np.sqrt(C * K * K)).astype(np.float32)
    bv = (rng.standard_normal(C) * 0.05).astype(np.float32)
    gv = (1 + 0.05 * rng.standard_normal(C)).astype(np.float32)
    btv = (0.05 * rng.standard_normal(C)).astype(np.float32)
    o = kernel(xs, wv, bv, gv, btv)
    print(o.shape, o.dtype, float(o.mean()))


# revision 12
# speedup vs baseline: 1.0074x; 1.0074x over previous
"""Trainium2 Bass kernel for nn_ModelNew_3556232921872 (dense_cnn).

Pipeline per sample:
  x_conv = conv3x3(x, W) + b
  acc    = depthwise3x3(x_conv, diag(W)) + b
  group stats over channels per pixel -> norm = (acc - mean_c) * rsqrt(var+eps)
  norm = norm * gamma + beta
  fused = tanh(norm) * clip(norm/6 + 0.5, 0, 1)
  out   = logsumexp(x_conv + fused, channels)          # [1, H, W]

Sharding: data-parallel over batch, B=16 -> 2 samples per NeuronCore x 8.

Implementation notes:
 - conv as 6 matmul passes: 3 passes K=128 (tap pairs (dy,0)+(dy,1) via a
   column-shifted second SBUF copy of the input) + 3 passes K=64 (taps (dy,2)),
   accumulated in PSUM. Depthwise conv: 9 passes K=64 with diagonal lhsT; the
   per-pixel channel-group mean is folded into the center tap
   (diag(wd) - G/8), so the DW psum directly yields t1 = acc - mean_c + bias.
 - Single ACT table set (natural_log_exp_and_others): rsqrt(v)=exp(-0.5 ln v),
   tanh(x)=1-2/(1+exp(2x)) with reciprocal_approx_fast on DVE, final ln on ACT.
"""
import numpy as np

import concourse.bass as bass
import concourse.bacc as bacc
import concourse.mybir as mybir
from concourse.tile import TileContext
from concourse.bass_utils import run_bass_kernel_spmd
from concourse.mybir import AluOpType, ActivationFunctionType

F32 = mybir.dt.float32
AF = ActivationFunctionType

B, C, H, W = 16, 64, 256, 256
K = 3
G = 8
GS = C // G
EPS = 1e-05
NCORES = 8
BPC = B // NCORES          # samples per core

R = 4                      # output rows per block
WP = W + 4                 # padded input row width (2 left, 2 right)
WO = W + 2                 # conv output row width (x in [-1, W+1))
NBLK = H // R

MAX_N_F32 = 512            # fp32 moving-operand limit per matmul


def _build_nc():
    import os as _os
    KSTAGE = int(_os.environ.get("KSTAGE", "0"))
    nc = bacc.Bacc("TRN2", target_bir_lowering=False)
    x = nc.dram_tensor("x", [BPC, 128, H + 4, WP], F32, kind="ExternalInput")
    wp = nc.dram_tensor("wp", [128, 3, C], F32, kind="ExternalInput")
    ws = nc.dram_tensor("ws", [64, 3, C], F32, kind="ExternalInput")
    wd = nc.dram_tensor("wd", [64, 9, C], F32, kind="ExternalInput")
    wstat = nc.dram_tensor("wstat", [128, 128], F32, kind="ExternalInput")
    ones = nc.dram_tensor("ones", [64, 1], F32, kind="ExternalInput")
    cb = nc.dram_tensor("cb", [64, 1], F32, kind="ExternalInput")
    gam = nc.dram_tensor("gam", [64, 1], F32, kind="ExternalInput")
    bet = nc.dram_tensor("bet", [64, 1], F32, kind="ExternalInput")
    out = nc.dram_tensor("out", [BPC, H * W], F32, kind="ExternalOutput")

    NRI = R + 4            # input rows per block (conv computes R+2 out rows)
    NRO = R + 2            # conv out rows [-1, R+1)
    NPIX = R * W

    with TileContext(nc) as tc:
        with tc.tile_pool(name="consts", bufs=1) as consts, \
             tc.tile_pool(name="xin_p", bufs=2) as xin_p, \
             tc.tile_pool(name="xcv_p", bufs=2) as xcv_p, \
             tc.tile_pool(name="work", bufs=2) as work, \
             tc.tile_pool(name="outp", bufs=2) as outp, \
             tc.tile_pool(name="psbig", bufs=1, space="PSUM") as psbig, \
             tc.tile_pool(name="psa", bufs=2, space="PSUM") as psa:

            wpt = consts.tile([128, 3, C], F32)
            wst = consts.tile([64, 3, C], F32)
            wdt = consts.tile([64, 9, C], F32)
            wstt = consts.tile([128, 128], F32)
            onest = consts.tile([64, 1], F32)
            cbt = consts.tile([64, 1], F32)
            gamt = consts.tile([64, 1], F32)
            bett = consts.tile([64, 1], F32)
            nc.sync.dma_start(out=wpt, in_=wp[:, :, :])
            nc.sync.dma_start(out=wst, in_=ws[:, :, :])
            nc.sync.dma_start(out=wdt, in_=wd[:, :, :])
            nc.sync.dma_start(out=wstt, in_=wstat[:, :])
            nc.sync.dma_start(out=onest, in_=ones[:, :])
            nc.sync.dma_start(out=cbt, in_=cb[:, :])
            nc.sync.dma_start(out=gamt, in_=gam[:, :])
            nc.sync.dma_start(out=bett, in_=bet[:, :])

            for b in range(BPC):
                for blk in range(NBLK):
                    y0 = blk * R
                    # ---- input tile [128, NRI, WP]; bottom half col-shifted +1
                    xin = xin_p.tile([128, NRI, WP], F32, tag="xin")
                    nc.sync.dma_start(out=xin, in_=x[b, :, y0:y0 + NRI, :])

                    # ---- conv psum [64, NRO, W]: rows y in [y0-1, y0+R+1),
                    # x in [0, W) only (column halos of xcv are SAME-pad zeros)
                    pc = psbig.tile([64, NRO, W], F32, tag="psbig")
                    for dy in range(3):
                        for j in range(0, NRO, 2):
                            rhs = bass.AP(
                                tensor=xin.tensor,
                                offset=xin.offset + (dy + j) * WP + 2,
                                ap=[[xin.ap[0][0], 128], [WP, 2], [1, W]])
                            nc.tensor.matmul(pc[:, j:j + 2, :], wpt[:, dy, :],
                                             rhs, start=(dy == 0), stop=False)
                    for dy in range(3):
                        for j in range(0, NRO, 2):
                            rhs = bass.AP(
                                tensor=xin.tensor,
                                offset=xin.offset + (dy + j) * WP + 3,
                                ap=[[xin.ap[0][0], 64], [WP, 2], [1, W]])
                            nc.tensor.matmul(pc[:, j:j + 2, :], wst[:, dy, :],
                                             rhs, start=False,
                                             stop=(dy == 2 and j == NRO - 2))

                    # ---- xcv [128, NRO, WO]: top = x_conv (zero halo), bottom = x_conv^2
                    xcv = xcv_p.tile([128, NRO, WO], F32, tag="xcv")
                    nc.vector.memset(xcv[0:64, :, 0:1], 0.0)
                    nc.vector.memset(xcv[0:64, :, WO - 1:WO], 0.0)
                    j_lo = 1 if blk == 0 else 0
                    j_hi = R + 1 if blk == NBLK - 1 else R + 2
                    if j_lo > 0:
                        nc.vector.memset(xcv[0:64, 0:j_lo, 1:WO - 1], 0.0)
                    if j_hi < NRO:
                        nc.vector.memset(xcv[0:64, j_hi:NRO, 1:WO - 1], 0.0)
                    nc.scalar.activation(xcv[0:64, j_lo:j_hi, 1:WO - 1],
                                         pc[:, j_lo:j_hi, :], AF.Identity,
                                         bias=cbt, scale=1.0)
                    nc.scalar.activation(xcv[64:128], xcv[0:64], AF.Square)

                    if KSTAGE == 1:
                        lse = outp.tile([1, R, W], F32, tag="lse")
                        nc.vector.tensor_copy(lse, xcv[0:1, 1:R + 1, 1:WO - 1])
                        nc.sync.dma_start(
                            out=out[b, y0 * W:(y0 + R) * W].rearrange("(o a c) -> o a c", o=1, c=W),
                            in_=lse)
                        continue
                    # ---- depthwise psum -> t1 = acc - mean_c (center folded)
                    pd = psa.tile([64, R, W], F32, tag="psa")
                    t = 0
                    for dy in range(3):
                        for dx in range(3):
                            for j0r in range(0, R, 2):
                                rhs = bass.AP(
                                    tensor=xcv.tensor,
                                    offset=xcv.offset + (dy + j0r) * WO + dx,
                                    ap=[[xcv.ap[0][0], 64], [WO, 2], [1, W]])
                                nc.tensor.matmul(
                                    pd[:, j0r:j0r + 2, :], wdt[:, t, :], rhs,
                                    start=(t == 0),
                                    stop=(t == 8 and j0r == R - 2))
                            t += 1
                    t1 = work.tile([64, R, W], F32, tag="t1")
                    nc.scalar.activation(t1, pd, AF.Identity, bias=cbt, scale=1.0)

                    # ---- stats psum [128, R, W]: mean_c / meansq_c
                    pstat = psbig.tile([128, R, W], F32, tag="psbig")
                    for j0r in range(0, R, 2):
                        rhs = bass.AP(
                            tensor=xcv.tensor,
                            offset=xcv.offset + (1 + j0r) * WO + 1,
                            ap=[[xcv.ap[0][0], 128], [WO, 2], [1, W]])
                        nc.tensor.matmul(pstat[:, j0r:j0r + 2, :], wstt, rhs,
                                         start=True, stop=(j0r == R - 2))
                                        # inv_std = exp(-0.5 * ln(meansq - mean^2 + eps))
                    m2 = work.tile([64, R, W], F32, tag="m2")
                    nc.scalar.activation(m2, pstat[0:64], AF.Square)
                    veps = work.tile([64, R, W], F32, tag="veps")
                    nc.vector.scalar_tensor_tensor(
                        out=veps, in0=pstat[64:128], scalar=EPS, in1=m2,
                        op0=AluOpType.add, op1=AluOpType.subtract)
                    if KSTAGE == 2:
                        lse = outp.tile([1, R, W], F32, tag="lse")
                        nc.vector.tensor_copy(lse, veps[0:1])
                        nc.sync.dma_start(
                            out=out[b, y0 * W:(y0 + R) * W].rearrange("(o a c) -> o a c", o=1, c=W),
                            in_=lse)
                        continue
                    lnv = work.tile([64, R, W], F32, tag="lnv")
                    nc.scalar.activation(lnv, veps, AF.Ln)
                    isd = work.tile([64, R, W], F32, tag="isd")
                    nc.scalar.activation(isd, lnv, AF.Exp, scale=-0.5)

                    # ---- norm
                    nrm = work.tile([64, R, W], F32, tag="nrm")
                    nc.vector.tensor_tensor(nrm, t1, isd, op=AluOpType.mult)
                    nc.vector.tensor_scalar(nrm, nrm, gamt, bett,
                                            AluOpType.mult, AluOpType.add)
                    nc.vector.tensor_scalar(nrm, nrm, -30.0, 30.0,
                                            AluOpType.max, AluOpType.min)

                    if KSTAGE == 3:
                        lse = outp.tile([1, R, W], F32, tag="lse")
                        nc.vector.tensor_copy(lse, nrm[0:1])
                        nc.sync.dma_start(
                            out=out[b, y0 * W:(y0 + R) * W].rearrange("(o a c) -> o a c", o=1, c=W),
                            in_=lse)
                        continue
                    # gate = clip(nrm/6 + 0.5, 0, 1)
                    gate = work.tile([64, R, W], F32, tag="gate")
                    nc.vector.tensor_scalar(gate, nrm, 1.0 / 6.0, 0.5,
                                            AluOpType.mult, AluOpType.add)
                    nc.vector.tensor_scalar(gate, gate, 0.0, 1.0,
                                            AluOpType.max, AluOpType.min)

                    # tanh(nrm) = 1 - 2/(1 + exp(2*nrm))
                    ee = work.tile([64, R, W], F32, tag="ee")
                    nc.scalar.activation(ee, nrm, AF.Exp, scale=2.0)
                    nc.vector.tensor_scalar(ee, ee, 1.0, None, AluOpType.add)
                    rr = work.tile([64, R, W], F32, tag="rr")
                    nc.vector.reciprocal_approx_fast(rr, ee)
                    # fused = (1 - 2 r) * gate = -2*(r*gate) + gate
                    nc.vector.tensor_tensor(rr, rr, gate, op=AluOpType.mult)
                    zz = work.tile([64, R, W], F32, tag="zz")
                    nc.vector.scalar_tensor_tensor(
                        out=zz, in0=rr, scalar=-2.0, in1=gate,
                        op0=AluOpType.mult, op1=AluOpType.add)
                    # z = x_conv + fused
                    xcv_int = bass.AP(
                        tensor=xcv.tensor, offset=xcv.offset + WO + 1,
                        ap=[[xcv.ap[0][0], 64], [WO, R], [1, W]])
                    nc.vector.tensor_tensor(zz, zz, xcv_int,
                                            op=AluOpType.add)
                    nc.scalar.activation(zz, zz, AF.Exp)

                    # ---- logsumexp: PE channel sum then ln
                    pl = psa.tile([1, R, W], F32, tag="psa")
                    for j0r in range(0, R, 2):
                        nc.tensor.matmul(
                            pl[:, j0r:j0r + 2, :], onest,
                            zz[:, j0r:j0r + 2, :],
                            start=True, stop=(j0r == R - 2))
                    lse = outp.tile([1, R, W], F32, tag="lse")
                    nc.scalar.activation(lse, pl, AF.Ln)
                    nc.sync.dma_start(
                        out=out[b, y0 * W:(y0 + R) * W].rearrange("(o a c) -> o a c", o=1, c=W),
                        in_=lse)
    nc.compile()
    return nc


def _host_weights(conv_w, conv_b, gn_scale, gn_bias):
    w = np.asarray(conv_w, np.float32)
    wp = np.stack([np.concatenate([w[:, :, dy, 1].T, w[:, :, dy, 0].T], axis=0)
                   for dy in range(3)], axis=1).astype(np.float32)
    ws = np.stack([w[:, :, dy, 2].T for dy in range(3)], axis=1).astype(np.float32)

    wdiag = np.einsum('cckl->ckl', w)                       # [C, 3, 3]
    gsel = np.zeros((C, C), np.float32)
    for g in range(G):
        gsel[g * GS:(g + 1) * GS, g * GS:(g + 1) * GS] = 1.0 / GS
    dmats = []
    for dy in range(3):
        for dx in range(3):
            m = np.diag(wdiag[:, dy, dx]).astype(np.float32)
            if dy == 1 and dx == 1:
                m = m - gsel            # fold -mean_c (lhsT[ci,co]: G sym)
            dmats.append(m)
    wd = np.stack(dmats, axis=1).astype(np.float32)          # [64, 9, 64]

    wstat = np.zeros((128, 128), np.float32)
    wstat[0:64, 0:64] = gsel
    wstat[64:128, 64:128] = gsel

    ones = np.ones((64, 1), np.float32)
    cb = np.asarray(conv_b, np.float32).reshape(64, 1)
    gam = np.asarray(gn_scale, np.float32).reshape(64, 1)
    bet = np.asarray(gn_bias, np.float32).reshape(64, 1)
    return dict(wp=wp, ws=ws, wd=wd, wstat=wstat, ones=ones,
                cb=cb, gam=gam, bet=bet)


_NC_CACHE = None


def kernel(x, conv_w, conv_b, gn_scale, gn_bias):
    global _NC_CACHE
    x = np.asarray(x, np.float32)
    wts = _host_weights(conv_w, conv_b, gn_scale, gn_bias)
    if _NC_CACHE is None:
        _NC_CACHE = _build_nc()
    nc = _NC_CACHE
    xpad = np.zeros((B, 128, H + 4, WP), np.float32)
    xpad[:, 0:64, 2:2 + H, 2:2 + W] = x
    xpad[:, 64:128, 2:2 + H, 3:3 + W] = x
    in_maps = []
    for c in range(NCORES):
        m = {"x": np.ascontiguousarray(xpad[c * BPC:(c + 1) * BPC])}
        m.update(wts)
        in_maps.append(m)
    import os as _os
    trace = bool(int(_os.environ.get("KTRACE", "0")))
    res = run_bass_kernel_spmd(nc, in_maps, core_ids=list(range(NCORES)),
                               trace=trace)
    kernel.exec_time_ns = res.exec_time_ns
    kernel.results_obj = res
    outs = [res.results[c]["out"].reshape(BPC, 1, H, W) for c in range(NCORES)]
    return np.concatenate(outs, axis=0)


if __name__ == "__main__":
    rng = np.random.default_rng(0)
    xs = rng.standard_normal((B, C, H, W), dtype=np.float32)
    wv = (rng.standard_normal((C, C, K, K), dtype=np.float32)
          / np.sqrt(C * K * K)).astype(np.float32)
    bv = (rng.standard_normal(C) * 0.05).astype(np.float32)
    gv = (1 + 0.05 * rng.standard_normal(C)).astype(np.float32)
    btv = (0.05 * rng.standard_normal(C)).astype(np.float32)
    o = kernel(xs, wv, bv, gv, btv)
    print(o.shape, o.dtype, float(o.mean()))


# revision 13
# speedup vs baseline: 12.9723x; 12.8768x over previous
"""Trainium2 Bass kernel for nn_ModelNew_3556232921872 (dense_cnn).

Pipeline per sample:
  x_conv = conv3x3(x, W) + b
  acc    = depthwise3x3(x_conv, diag(W)) + b
  group stats over channels per pixel -> norm = (acc - mean_c) * rsqrt(var+eps)
  norm = norm * gamma + beta
  fused = tanh(norm) * clip(norm/6 + 0.5, 0, 1)
  out   = logsumexp(x_conv + fused, channels)          # [1, H, W]

Sharding: data-parallel over batch, B=16 -> 2 samples per NeuronCore x 8.

Implementation notes:
 - conv as 6 matmul passes: 3 passes K=128 (tap pairs (dy,0)+(dy,1) via a
   column-shifted second SBUF copy of the input) + 3 passes K=64 (taps (dy,2)),
   accumulated in PSUM. Depthwise conv: 9 passes K=64 with diagonal lhsT; the
   per-pixel channel-group mean is folded into the center tap
   (diag(wd) - G/8), so the DW psum directly yields t1 = acc - mean_c + bias.
 - Single ACT table set (natural_log_exp_and_others): rsqrt(v)=exp(-0.5 ln v),
   tanh(x)=1-2/(1+exp(2x)) with reciprocal_approx_fast on DVE, final ln on ACT.
"""
import numpy as np

import concourse.bass as bass
import concourse.bacc as bacc
import concourse.mybir as mybir
from concourse.tile import TileContext
from concourse.bass_utils import run_bass_kernel_spmd
from concourse.mybir import AluOpType, ActivationFunctionType

F32 = mybir.dt.float32
AF = ActivationFunctionType

B, C, H, W = 16, 64, 256, 256
K = 3
G = 8
GS = C // G
EPS = 1e-05
NCORES = 8
BPC = B // NCORES          # samples per core

R = 4                      # output rows per block
WP = W + 4                 # padded input row width (2 left, 2 right)
WO = W + 2                 # conv output row width (x in [-1, W+1))
NBLK = H // R

MAX_N_F32 = 512            # fp32 moving-operand limit per matmul


def _build_nc():
    import os as _os
    KSTAGE = int(_os.environ.get("KSTAGE", "0"))
    nc = bacc.Bacc("TRN2", target_bir_lowering=False)
    x = nc.dram_tensor("x", [BPC, 128, H + 4, WP], F32, kind="ExternalInput")
    wp = nc.dram_tensor("wp", [128, 3, C], F32, kind="ExternalInput")
    ws = nc.dram_tensor("ws", [64, 3, C], F32, kind="ExternalInput")
    wd = nc.dram_tensor("wd", [64, 9, C], F32, kind="ExternalInput")
    wstat = nc.dram_tensor("wstat", [128, 128], F32, kind="ExternalInput")
    ones = nc.dram_tensor("ones", [64, 1], F32, kind="ExternalInput")
    cb = nc.dram_tensor("cb", [64, 1], F32, kind="ExternalInput")
    gam = nc.dram_tensor("gam", [64, 1], F32, kind="ExternalInput")
    bet = nc.dram_tensor("bet", [64, 1], F32, kind="ExternalInput")
    out = nc.dram_tensor("out", [BPC, H * W], F32, kind="ExternalOutput")

    NRI = R + 4            # input rows per block (conv computes R+2 out rows)
    NRO = R + 2            # conv out rows [-1, R+1)
    NPIX = R * W

    with TileContext(nc) as tc:
        with tc.tile_pool(name="consts", bufs=1) as consts, \
             tc.tile_pool(name="xin_p", bufs=2) as xin_p, \
             tc.tile_pool(name="xcv_p", bufs=2) as xcv_p, \
             tc.tile_pool(name="work", bufs=2) as work, \
             tc.tile_pool(name="outp", bufs=2) as outp, \
             tc.tile_pool(name="psbig", bufs=1, space="PSUM") as psbig, \
             tc.tile_pool(name="psa", bufs=2, space="PSUM") as psa:

            wpt = consts.tile([128, 3, C], F32)
            wst = consts.tile([64, 3, C], F32)
            wdt = consts.tile([64, 9, C], F32)
            wstt = consts.tile([128, 128], F32)
            onest = consts.tile([64, 1], F32)
            cbt = consts.tile([64, 1], F32)
            gamt = consts.tile([64, 1], F32)
            bett = consts.tile([64, 1], F32)
            nc.sync.dma_start(out=wpt, in_=wp[:, :, :])
            nc.sync.dma_start(out=wst, in_=ws[:, :, :])
            nc.sync.dma_start(out=wdt, in_=wd[:, :, :])
            nc.sync.dma_start(out=wstt, in_=wstat[:, :])
            nc.sync.dma_start(out=onest, in_=ones[:, :])
            nc.sync.dma_start(out=cbt, in_=cb[:, :])
            nc.sync.dma_start(out=gamt, in_=gam[:, :])
            nc.sync.dma_start(out=bett, in_=bet[:, :])

            KREP = int(_os.environ.get("KREPEAT", "1"))
            for _rep in range(KREP):
              for b in range(BPC):
                for blk in range(NBLK):
                    y0 = blk * R
                    # ---- input tile [128, NRI, WP]; bottom half col-shifted +1
                    xin = xin_p.tile([128, NRI, WP], F32, tag="xin")
                    nc.sync.dma_start(out=xin, in_=x[b, :, y0:y0 + NRI, :])

                    # ---- conv psum [64, NRO, W]: rows y in [y0-1, y0+R+1),
                    # x in [0, W) only (column halos of xcv are SAME-pad zeros)
                    pc = psbig.tile([64, NRO, W], F32, tag="psbig")
                    for dy in range(3):
                        for j in range(0, NRO, 2):
                            rhs = bass.AP(
                                tensor=xin.tensor,
                                offset=xin.offset + (dy + j) * WP + 2,
                                ap=[[xin.ap[0][0], 128], [WP, 2], [1, W]])
                            nc.tensor.matmul(pc[:, j:j + 2, :], wpt[:, dy, :],
                                             rhs, start=(dy == 0), stop=False)
                    for dy in range(3):
                        for j in range(0, NRO, 2):
                            rhs = bass.AP(
                                tensor=xin.tensor,
                                offset=xin.offset + (dy + j) * WP + 3,
                                ap=[[xin.ap[0][0], 64], [WP, 2], [1, W]])
                            nc.tensor.matmul(pc[:, j:j + 2, :], wst[:, dy, :],
                                             rhs, start=False,
                                             stop=(dy == 2 and j == NRO - 2))

                    # ---- xcv [128, NRO, WO]: top = x_conv (zero halo), bottom = x_conv^2
                    xcv = xcv_p.tile([128, NRO, WO], F32, tag="xcv")
                    nc.vector.memset(xcv[0:64, :, 0:1], 0.0)
                    nc.vector.memset(xcv[0:64, :, WO - 1:WO], 0.0)
                    j_lo = 1 if blk == 0 else 0
                    j_hi = R + 1 if blk == NBLK - 1 else R + 2
                    if j_lo > 0:
                        nc.vector.memset(xcv[0:64, 0:j_lo, 1:WO - 1], 0.0)
                    if j_hi < NRO:
                        nc.vector.memset(xcv[0:64, j_hi:NRO, 1:WO - 1], 0.0)
                    nc.scalar.activation(xcv[0:64, j_lo:j_hi, 1:WO - 1],
                                         pc[:, j_lo:j_hi, :], AF.Identity,
                                         bias=cbt, scale=1.0)
                    nc.scalar.activation(xcv[64:128], xcv[0:64], AF.Square)

                    if KSTAGE == 1:
                        lse = outp.tile([1, R, W], F32, tag="lse")
                        nc.vector.tensor_copy(lse, xcv[0:1, 1:R + 1, 1:WO - 1])
                        nc.sync.dma_start(
                            out=out[b, y0 * W:(y0 + R) * W].rearrange("(o a c) -> o a c", o=1, c=W),
                            in_=lse)
                        continue
                    # ---- depthwise psum -> t1 = acc - mean_c (center folded)
                    pd = psa.tile([64, R, W], F32, tag="psa")
                    t = 0
                    for dy in range(3):
                        for dx in range(3):
                            for j0r in range(0, R, 2):
                                rhs = bass.AP(
                                    tensor=xcv.tensor,
                                    offset=xcv.offset + (dy + j0r) * WO + dx,
                                    ap=[[xcv.ap[0][0], 64], [WO, 2], [1, W]])
                                nc.tensor.matmul(
                                    pd[:, j0r:j0r + 2, :], wdt[:, t, :], rhs,
                                    start=(t == 0),
                                    stop=(t == 8 and j0r == R - 2))
                            t += 1
                    t1 = work.tile([64, R, W], F32, tag="t1")
                    nc.scalar.activation(t1, pd, AF.Identity, bias=cbt, scale=1.0)

                    # ---- stats psum [128, R, W]: mean_c / meansq_c
                    pstat = psbig.tile([128, R, W], F32, tag="psbig")
                    for j0r in range(0, R, 2):
                        rhs = bass.AP(
                            tensor=xcv.tensor,
                            offset=xcv.offset + (1 + j0r) * WO + 1,
                            ap=[[xcv.ap[0][0], 128], [WO, 2], [1, W]])
                        nc.tensor.matmul(pstat[:, j0r:j0r + 2, :], wstt, rhs,
                                         start=True, stop=(j0r == R - 2))
                                        # inv_std = exp(-0.5 * ln(meansq - mean^2 + eps))
                    m2 = work.tile([64, R, W], F32, tag="m2")
                    nc.scalar.activation(m2, pstat[0:64], AF.Square)
                    veps = work.tile([64, R, W], F32, tag="veps")
                    nc.vector.scalar_tensor_tensor(
                        out=veps, in0=pstat[64:128], scalar=EPS, in1=m2,
                        op0=AluOpType.add, op1=AluOpType.subtract)
                    if KSTAGE == 2:
                        lse = outp.tile([1, R, W], F32, tag="lse")
                        nc.vector.tensor_copy(lse, veps[0:1])
                        nc.sync.dma_start(
                            out=out[b, y0 * W:(y0 + R) * W].rearrange("(o a c) -> o a c", o=1, c=W),
                            in_=lse)
                        continue
                    lnv = work.tile([64, R, W], F32, tag="lnv")
                    nc.scalar.activation(lnv, veps, AF.Ln)
                    isd = work.tile([64, R, W], F32, tag="isd")
                    nc.scalar.activation(isd, lnv, AF.Exp, scale=-0.5)

                    # ---- norm
                    nrm = work.tile([64, R, W], F32, tag="nrm")
                    nc.vector.tensor_tensor(nrm, t1, isd, op=AluOpType.mult)
                    nc.vector.tensor_scalar(nrm, nrm, gamt, bett,
                                            AluOpType.mult, AluOpType.add)
                    nc.vector.tensor_scalar(nrm, nrm, -30.0, 30.0,
                                            AluOpType.max, AluOpType.min)

                    if KSTAGE == 3:
                        lse = outp.tile([1, R, W], F32, tag="lse")
                        nc.vector.tensor_copy(lse, nrm[0:1])
                        nc.sync.dma_start(
                            out=out[b, y0 * W:(y0 + R) * W].rearrange("(o a c) -> o a c", o=1, c=W),
                            in_=lse)
                        continue
                    # gate = clip(nrm/6 + 0.5, 0, 1)
                    gate = work.tile([64, R, W], F32, tag="gate")
                    nc.vector.tensor_scalar(gate, nrm, 1.0 / 6.0, 0.5,
                                            AluOpType.mult, AluOpType.add)
                    nc.vector.tensor_scalar(gate, gate, 0.0, 1.0,
                                            AluOpType.max, AluOpType.min)

                    # tanh(nrm) = 1 - 2/(1 + exp(2*nrm))
                    ee = work.tile([64, R, W], F32, tag="ee")
                    nc.scalar.activation(ee, nrm, AF.Exp, scale=2.0)
                    nc.vector.tensor_scalar(ee, ee, 1.0, None, AluOpType.add)
                    rr = work.tile([64, R, W], F32, tag="rr")
                    nc.vector.reciprocal_approx_fast(rr, ee)
                    # fused = (1 - 2 r) * gate = -2*(r*gate) + gate
                    nc.vector.tensor_tensor(rr, rr, gate, op=AluOpType.mult)
                    zz = work.tile([64, R, W], F32, tag="zz")
                    nc.vector.scalar_tensor_tensor(
                        out=zz, in0=rr, scalar=-2.0, in1=gate,
                        op0=AluOpType.mult, op1=AluOpType.add)
                    # z = x_conv + fused
                    xcv_int = bass.AP(
                        tensor=xcv.tensor, offset=xcv.offset + WO + 1,
                        ap=[[xcv.ap[0][0], 64], [WO, R], [1, W]])
                    nc.vector.tensor_tensor(zz, zz, xcv_int,
                                            op=AluOpType.add)
                    nc.scalar.activation(zz, zz, AF.Exp)

                    # ---- logsumexp: PE channel sum then ln
                    pl = psa.tile([1, R, W], F32, tag="psa")
                    for j0r in range(0, R, 2):
                        nc.tensor.matmul(
                            pl[:, j0r:j0r + 2, :], onest,
                            zz[:, j0r:j0r + 2, :],
                            start=True, stop=(j0r == R - 2))
                    lse = outp.tile([1, R, W], F32, tag="lse")
                    nc.scalar.activation(lse, pl, AF.Ln)
                    nc.sync.dma_start(
                        out=out[b, y0 * W:(y0 + R) * W].rearrange("(o a c) -> o a c", o=1, c=W),
                        in_=lse)
    nc.compile()
    return nc


def _host_weights(conv_w, conv_b, gn_scale, gn_bias):
    w = np.asarray(conv_w, np.float32)
    wp = np.stack([np.concatenate([w[:, :, dy, 1].T, w[:, :, dy, 0].T], axis=0)
                   for dy in range(3)], axis=1).astype(np.float32)
    ws = np.stack([w[:, :, dy, 2].T for dy in range(3)], axis=1).astype(np.float32)

    wdiag = np.einsum('cckl->ckl', w)                       # [C, 3, 3]
    gsel = np.zeros((C, C), np.float32)
    for g in range(G):
        gsel[g * GS:(g + 1) * GS, g * GS:(g + 1) * GS] = 1.0 / GS
    dmats = []
    for dy in range(3):
        for dx in range(3):
            m = np.diag(wdiag[:, dy, dx]).astype(np.float32)
            if dy == 1 and dx == 1:
                m = m - gsel            # fold -mean_c (lhsT[ci,co]: G sym)
            dmats.append(m)
    wd = np.stack(dmats, axis=1).astype(np.float32)          # [64, 9, 64]

    wstat = np.zeros((128, 128), np.float32)
    wstat[0:64, 0:64] = gsel
    wstat[64:128, 64:128] = gsel

    ones = np.ones((64, 1), np.float32)
    cb = np.asarray(conv_b, np.float32).reshape(64, 1)
    gam = np.asarray(gn_scale, np.float32).reshape(64, 1)
    bet = np.asarray(gn_bias, np.float32).reshape(64, 1)
    return dict(wp=wp, ws=ws, wd=wd, wstat=wstat, ones=ones,
                cb=cb, gam=gam, bet=bet)


_NC_CACHE = None


def kernel(x, conv_w, conv_b, gn_scale, gn_bias):
    global _NC_CACHE
    x = np.asarray(x, np.float32)
    wts = _host_weights(conv_w, conv_b, gn_scale, gn_bias)
    if _NC_CACHE is None:
        _NC_CACHE = _build_nc()
    nc = _NC_CACHE
    xpad = np.zeros((B, 128, H + 4, WP), np.float32)
    xpad[:, 0:64, 2:2 + H, 2:2 + W] = x
    xpad[:, 64:128, 2:2 + H, 3:3 + W] = x
    in_maps = []
    for c in range(NCORES):
        m = {"x": np.ascontiguousarray(xpad[c * BPC:(c + 1) * BPC])}
        m.update(wts)
        in_maps.append(m)
    import os as _os
    trace = bool(int(_os.environ.get("KTRACE", "0")))
    res = run_bass_kernel_spmd(nc, in_maps, core_ids=list(range(NCORES)),
                               trace=trace)
    kernel.exec_time_ns = res.exec_time_ns
    kernel.results_obj = res
    outs = [res.results[c]["out"].reshape(BPC, 1, H, W) for c in range(NCORES)]
    return np.concatenate(outs, axis=0)


if __name__ == "__main__":
    rng = np.random.default_rng(0)
    xs = rng.standard_normal((B, C, H, W), dtype=np.float32)
    wv = (rng.standard_normal((C, C, K, K), dtype=np.float32)
          / np.sqrt(C * K * K)).astype(np.float32)
    bv = (rng.standard_normal(C) * 0.05).astype(np.float32)
    gv = (1 + 0.05 * rng.standard_normal(C)).astype(np.float32)
    btv = (0.05 * rng.standard_normal(C)).astype(np.float32)
    o = kernel(xs, wv, bv, gv, btv)
    print(o.shape, o.dtype, float(o.mean()))
